# revision 29
# baseline (speedup 1.0000x reference)
"""Trainium2 Bass kernel for nn_GCNNDiagGaussianActor.

Key structural insight: the reference GNN runs GCNConv layers over a COMPLETE
graph of 32 nodes per sample with self-loops. Every node therefore has degree
exactly 32 and the symmetric GCN normalization is the constant 1/32 for every
edge. The gather + segment_sum message passing collapses to a per-graph mean
over nodes, broadcast back to every node. The whole network reduces to, per
graph g:

    pooled = sum_n obs[g, n, 2:16]                  (node-mean fused into W1)
    h1  = relu(pooled @ (W1 / 32) + b1)
    h2  = relu(h1 @ W2 + b2)
    m   = relu(h2 @ Wm1 + bm1)
    o   = m @ Wm2 + bm2                              -> [4] per graph
    mu  = o[:2];  std = exp(3.5 * tanh(o[2:]) - 1.5)
    out[0, g] = tile(mu, 32); out[1, g] = tile(std, 32)

Sharding: data-parallel over the batch. 1024 graphs / 8 cores = 128 graphs
per core = exactly the 128 SBUF partitions. Weights are replicated. The x32
node replication of the output is folded into the final matmul by replicating
Wm2's columns host-side.

Perf notes (v8, measured ~19.4us vs 23.5us fp32 baseline):
- all matmul operands in bf16: fp32 matmul costs 4 PE cycles/row (LOW+HIGH
  double pass) vs 1 for bf16; rel_err budget is 2e-2 and bf16 end-to-end
  sims at ~9e-4.
- obs DMA split across the two hardware DGE queues (sync/SP rows 0:112 +
  scalar/Act rows 112:128 — sync's ring starts ~0.5us earlier, so it gets
  the bulk), keeping 2KB packets; all weights ride the scalar queue in ONE
  packed bf16 tensor so they never serialize behind the full obs.
- node pooling via one strided tensor_reduce writing bf16 directly; then
  4 bf16 32x32 DVE block transposes (no PSUM round trip).
- relu+bias fused on DVE via tensor_scalar, bf16 out, fp32 PSUM stays exact.
- biases travel as bf16 columns of wpack and are upcast to an fp32 [128,5]
  tile off the critical path (they are exactly zero in this model anyway).
- final layer computes only the 4 unique per-graph outputs (N=4 matmul on
  the un-replicated Wm2); bm2 is added for all four columns by a K=1
  accumulating matmul (ones ⊗ bm2), so the log_std path needs just ONE
  unbiased tanh [128,2]; mu / std planes are replicated x32 on write via
  stride-0 broadcast access patterns (DVE copy / scalar EXP).
- the mu copy is emitted AFTER the activations so the tile scheduler does
  not hoist the out-DMA's wait-for-mu onto the scalar engine ahead of the
  tanh (cost ~350ns when it does).
- single [128, 128] output DMA (mu | std planes per row) issued from the
  scalar engine (EXP result is scalar-local: no cross-engine wake before
  the DGE setup); host splits the planes.
- dummy tanh after the scalar queue's DMA issues hoists the scalar engine's
  ACT_TABLE_LOAD (~1.3us) off the critical path.
"""

import numpy as np

NCORES = 8
BS = 1024
BS_LOCAL = BS // NCORES   # 128 graphs per core
NN = 32                   # nodes per graph
FD = 16                   # per-node obs width
OBS_W = NN * FD           # 512
H = 128                   # hidden width
OUT_W = 2 * NN            # 64 = ACT_DIM * NN
# wpack cols: W2 | Wm1 | Wm2 | b1 b2 bm1 bt0 bt1 | W1p (rows 0:16 only) |
# bm2 (row 0 only) | identity (for the PE transpose)
WPK = 2 * H + 4 + 5 + H + 4 + H

_NC_CACHE = {}


def _build_bass():
    import concourse.bacc as bacc
    import concourse.mybir as mybir
    from concourse import tile

    fp32 = mybir.dt.float32
    bf16 = mybir.dt.bfloat16
    AF = mybir.ActivationFunctionType
    ALU = mybir.AluOpType

    nc = bacc.Bacc(None, target_bir_lowering=False)
    obs = nc.declare_dram_parameter("obs", [BS_LOCAL, OBS_W], fp32, isOutput=False)
    # packed bf16: cols 0:128 W2 | 128:256 Wm1 | 256:384 Wm2r | 384 b1 |
    # 385 b2 | 386 bm1 | 387 bm2[2]*ones | 388 bm2[3]*ones
    wpack = nc.declare_dram_parameter("wpack", [H, WPK], bf16, isOutput=False)
    out = nc.declare_dram_parameter("out", [BS_LOCAL, 2 * OUT_W], fp32, isOutput=True)

    # obs is split by COLUMNS (node halves) across the two hardware DGE
    # queues so the reduce can start on the first half while the second is
    # still in flight; sync/SP's ring starts ~0.5us before scalar's, so it
    # carries the half whose reduce runs first.
    CH = OBS_W // 2  # 256 cols = nodes 0:16
    with tile.TileContext(nc) as tc:
        with (
            tc.tile_pool(name="sb", bufs=1) as pool,
            tc.tile_pool(name="ps", bufs=1, space="PSUM") as ppool,
        ):
            obs_t = pool.tile([BS_LOCAL, OBS_W], fp32)
            nc.sync.dma_start(obs_t[:, 0:CH], obs[:, 0:CH])
            nc.scalar.dma_start(obs_t[:, CH:OBS_W], obs[:, CH:OBS_W])
            wp = pool.tile([H, WPK], bf16)
            nc.scalar.dma_start(wp[:], wpack[:])
            w1b_t = wp[0:FD, 2 * H + 9 : 3 * H + 9]
            ident = wp[:, 3 * H + 13 : 4 * H + 13]

            cm15 = pool.tile([BS_LOCAL, 1], fp32)
            nc.vector.memset(cm15[:], -1.5)
            ones1 = pool.tile([1, BS_LOCAL], bf16)
            nc.vector.memset(ones1[:], 1.0)
            # dummy transcendental: hoists ACT_TABLE_LOAD into the DMA wait
            warm = pool.tile([1, 1], fp32)
            nc.vector.memset(warm[:], 0.0)
            nc.scalar.activation(warm[:], warm[:], AF.Tanh)

            # bf16 staging for pooled features (cols 0:2 are zeroed: the PE
            # transpose reads all 16 cols and W1 rows 0:2 are zeroed anyway).
            S = pool.tile([BS_LOCAL, FD], bf16)
            nc.vector.memset(S[:], 0.0)

            # Node pooling over the 14 used features: obs row is 32 node
            # blocks of 16; S[:, 2:16] = sum over nodes of cols 2:16.
            # Two half-reduces (nodes 0:16 / 16:32) so the first runs while
            # the second obs half is still transferring; fp32 partials, bf16
            # on the final add.
            Sa = pool.tile([BS_LOCAL, FD - 2], fp32)
            Sb = pool.tile([BS_LOCAL, FD - 2], fp32)
            nc.vector.tensor_reduce(
                Sa[:],
                obs_t[:, 0:CH].rearrange("p (n c) -> p c n", c=FD)[:, 2:FD, :],
                axis=mybir.AxisListType.X,
                op=ALU.add,
            )
            nc.vector.tensor_reduce(
                Sb[:],
                obs_t[:, CH:OBS_W].rearrange("p (n c) -> p c n", c=FD)[:, 2:FD, :],
                axis=mybir.AxisListType.X,
                op=ALU.add,
            )
            nc.vector.tensor_tensor(S[:, 2:FD], Sa[:], Sb[:], ALU.add)
            # fp32 biases for tensor_scalar / activation (scalar operands
            # must be fp32); upcast from wpack's bf16 tail columns off the
            # critical path (runs on DVE while the PE transposes S).
            biasf = pool.tile([H, 3], fp32)
            nc.vector.tensor_copy(biasf[:], wp[:, 2 * H + 4 : 2 * H + 7])

            # [128, 16] -> [16, 128] on the (otherwise idle) PE via
            # is_transpose with a DMA'd identity; DVE just copies the bf16
            # PSUM result back to SBUF for MM1's rhs.
            t_ps = ppool.tile([FD, BS_LOCAL], bf16)
            nc.tensor.transpose(t_ps[:], S[:], ident[:])
            T = pool.tile([FD, BS_LOCAL], bf16)
            nc.vector.tensor_copy(T[:], t_ps[:])

            # Channel-major MLP chain: [ch, graphs] bf16 tiles, fp32 PSUM,
            # relu+bias fused on DVE (out = max(psum + b, 0)).
            h1_ps = ppool.tile([H, BS_LOCAL], fp32)
            nc.tensor.matmul(h1_ps[:], w1b_t, T[:], start=True, stop=True)
            h1 = pool.tile([H, BS_LOCAL], bf16)
            nc.vector.tensor_scalar(
                h1[:], h1_ps[:], biasf[:, 0:1], 0.0, ALU.add, ALU.max
            )

            h2_ps = ppool.tile([H, BS_LOCAL], fp32)
            nc.tensor.matmul(h2_ps[:], wp[:, 0:H], h1[:], start=True, stop=True)
            h2 = pool.tile([H, BS_LOCAL], bf16)
            nc.vector.tensor_scalar(
                h2[:], h2_ps[:], biasf[:, 1:2], 0.0, ALU.add, ALU.max
            )

            m_ps = ppool.tile([H, BS_LOCAL], fp32)
            nc.tensor.matmul(m_ps[:], wp[:, H : 2 * H], h2[:], start=True, stop=True)
            m = pool.tile([H, BS_LOCAL], bf16)
            nc.vector.tensor_scalar(
                m[:], m_ps[:], biasf[:, 2:3], 0.0, ALU.add, ALU.max
            )

            # Final layer on the UN-replicated Wm2 [128, 4]: every node of a
            # graph shares the same 4 outputs, so compute them once (N=4) and
            # broadcast x32 on write via stride-0 access patterns.
            o_ps = ppool.tile([BS_LOCAL, 4], fp32)
            nc.tensor.matmul(
                o_ps[:], m[:], wp[:, 2 * H : 2 * H + 4], start=True, stop=False
            )
            # bm2 bias via a K=1 accumulating matmul (ones ⊗ bm2): both mu
            # and log_std columns come out of PSUM pre-biased.
            nc.tensor.matmul(
                o_ps[:], ones1[:], wp[0:1, 3 * H + 9 : 3 * H + 13],
                start=False, stop=True,
            )

            O = pool.tile([BS_LOCAL, 2 * OUT_W], fp32)
            # std = exp(3.5*tanh(ls) - 1.5), two unique columns; one EXP
            # reads the [128,2] tanh result broadcast x32 and writes the
            # full 64-wide plane.
            tls = pool.tile([BS_LOCAL, 2], fp32)
            nc.scalar.activation(tls[:], o_ps[:, 2:4], AF.Tanh)
            ls_src = tls[:].rearrange("p (n c) -> p n c", n=1).to_broadcast(
                [BS_LOCAL, NN, 2]
            )
            nc.scalar.activation(
                O[:, OUT_W : 2 * OUT_W].rearrange("p (n c) -> p n c", c=2),
                ls_src,
                AF.Exp,
                bias=cm15[:],
                scale=3.5,
            )
            # mu plane: replicate o_ps[:, 0:2] across the 32 nodes. Emitted
            # AFTER the activations so the scheduler doesn't hoist the out
            # DMA's wait-for-mu onto the scalar engine ahead of the tanhs.
            mu_src = o_ps[:, 0:2].rearrange("p (n c) -> p n c", n=1).to_broadcast(
                [BS_LOCAL, NN, 2]
            )
            nc.vector.tensor_copy(
                O[:, 0:OUT_W].rearrange("p (n c) -> p n c", c=2), mu_src
            )
            # issued from the scalar engine: the EXP result is scalar-local,
            # so no cross-engine wake sits before the DGE setup.
            nc.scalar.dma_start(out[:], O[:])

    nc.compile()
    return nc


def _get_nc():
    if "nc" not in _NC_CACHE:
        _NC_CACHE["nc"] = _build_bass()
    return _NC_CACHE["nc"]


def _prep_inputs(inputs):
    import ml_dtypes

    bf16 = ml_dtypes.bfloat16

    obs = np.ascontiguousarray(np.asarray(inputs["obs"], dtype=np.float32))
    W1 = np.asarray(inputs["W1"], dtype=np.float32)
    b1 = np.asarray(inputs["b1"], dtype=np.float32)
    W2 = np.asarray(inputs["W2"], dtype=np.float32)
    b2 = np.asarray(inputs["b2"], dtype=np.float32)
    Wm1 = np.asarray(inputs["Wm1"], dtype=np.float32)
    bm1 = np.asarray(inputs["bm1"], dtype=np.float32)
    Wm2 = np.asarray(inputs["Wm2"], dtype=np.float32)
    bm2 = np.asarray(inputs["bm2"], dtype=np.float32)

    d = np.float32(1.0) / np.float32(np.sqrt(np.float32(32.0)))
    norm2 = np.float32(d * d)              # GCN symmetric norm, all edges
    W1p = np.zeros((FD, H), np.float32)
    W1p[2:FD] = W1 * norm2                 # drops robot_loc cols 0:2, scales
    W2s = (W2 * np.float32(np.float32(32.0) * norm2)).astype(np.float32)

    ones = np.ones((H, 1), np.float32)
    w1rows = np.zeros((H, H), np.float32)
    w1rows[0:FD] = W1p                     # lhsT for layer 1 in rows 0:16
    bm2rows = np.zeros((H, 4), np.float32)
    bm2rows[0] = bm2                       # rhs row for the K=1 bias matmul
    wpack = np.ascontiguousarray(
        np.concatenate(
            [
                W2s,
                Wm1,
                Wm2,
                b1[:, None],
                b2[:, None],
                bm1[:, None],
                bm2[2] * ones,
                bm2[3] * ones,
                w1rows,
                bm2rows,
                np.eye(H, dtype=np.float32),
            ],
            axis=1,
        ).astype(bf16)
    )

    shared = {"wpack": wpack}
    in_maps = []
    for c in range(NCORES):
        mm = dict(shared)
        mm["obs"] = obs[c * BS_LOCAL : (c + 1) * BS_LOCAL]
        in_maps.append(mm)
    return in_maps


def kernel(**inputs):
    from concourse.bass_utils import run_bass_kernel_spmd

    assert inputs["obs"].shape == (BS, OBS_W), inputs["obs"].shape
    nc = _get_nc()
    in_maps = _prep_inputs(inputs)
    res = run_bass_kernel_spmd(nc, in_maps, list(range(NCORES))).results
    out = np.empty((2, BS, OUT_W), np.float32)
    for c in range(NCORES):
        o = res[c]["out"].reshape(BS_LOCAL, 2, OUT_W)
        out[0, c * BS_LOCAL : (c + 1) * BS_LOCAL, :] = o[:, 0, :]
        out[1, c * BS_LOCAL : (c + 1) * BS_LOCAL, :] = o[:, 1, :]
    return out
